# revision 17
# baseline (speedup 1.0000x reference)
"""Trainium2 Bass kernel for nn_BasicBlockOurIn (sparse-conv BasicBlock).

Computation (see problem reference):
    out = lrelu(inorm2(conv(lrelu(inorm1(conv(f, w1))), w2)) + f)
where conv is a 27-tap kernel-map sparse convolution, inorm is per-batch-
instance instance norm (unbiased var), lrelu slope 0.01.

Sharding: batch_ids are sorted with exactly 8192 points per instance and the
kernel map never crosses instances, so each of the 8 NeuronCores handles one
instance independently (no collectives).

Architecture (v2): everything stays feature-major [C, points] in SBUF; the
DRAM row-table round-trips, DMA gathers/scatters and DMA transposes of the
previous version are eliminated.

  - The center tap (identity permutation) is dense: W-stationary matmuls
    (lhsT=W_id, rhs=fT tile) emit the output feature-major in PSUM.
  - Non-identity taps are ~1% sparse.  Host compacts the valid (k,src,dst)
    triples into a token stream ordered by (dst_block512, tap, dst), padded
    per (block, tap) to the max over cores (SPMD-uniform layout):
      * conv1 gathers are host-side (g1 = feats[src] feature-major);
      * per-tap matmuls (lhsT=W_k) produce Y feature-major in PSUM;
      * PE transposes give Y in token-rows form;
      * a one-hot scatter matmul per token chunk accumulates Y into the
        dense PSUM tile of its dst block (host-built fp8 one-hot rhs).
        Duplicate dsts sum in PSUM for free.
  - The kernel map is symmetric (dst of every token is the src of its
    reverse token), so conv2 uses the *same* token stream.  Its gathered
    inputs are computed in token space, never from a full a1 row table:
      G2raw[:,t] = conv1[src(t)] = W_id1^T g1[:,t] + sum_{dst(t')=src(t)} Y1[:,t']
    The correction is a banded one-hot routing matmul (src ~ dst +- ~100
    positions), then G2 = lrelu(s1*G2raw + b1) on the scalar engine.
  - Instance-norm stats via bn_stats/bn_aggr on drained tiles; conv1 apply
    fused into one scalar-engine Lrelu; conv2 tail z = s2*x + ft split
    across PE (diag matmuls) / DVE / Pool, then Lrelu(z + b2) on scalar.
"""

import sys

if "/opt/trn_rl_repo" not in sys.path:
    sys.path.insert(0, "/opt/trn_rl_repo")

import numpy as np

N = 65536
C = 128
B = 8
PER = 8192
KVOL = 27
P = 128
NCORES = 8
EPS = 1e-6
NEG_SLOPE = 0.01
BLK = 512
NBLK = PER // BLK          # 16 dst blocks = 16 x 512-point tiles
DSTG = 1024                # dense/point-space stage width
NDST = PER // DSTG         # 8 dense stages

_plan_cache = {}
_prog_cache = {}


# --------------------------------------------------------------------------
# host-side planning
# --------------------------------------------------------------------------

def _build_plan(nbr):
    """Analyze neighbor_idx; None if the sharding assumptions fail."""
    arange_n = np.arange(N, dtype=np.int64)
    identity_ks = [k for k in range(KVOL)
                   if np.array_equal(nbr[k], arange_n)]

    loc = np.empty((NCORES, KVOL, PER), dtype=np.int64)
    valid = np.empty((NCORES, KVOL, PER), dtype=bool)
    for c in range(NCORES):
        sl = nbr[:, c * PER:(c + 1) * PER].astype(np.int64)
        v = sl >= 0
        l = sl - c * PER
        if ((l < 0) | (l >= PER))[v].any():
            return None  # non-local neighbor: fall back
        loc[c] = l
        valid[c] = v

    sp_ks = [k for k in range(KVOL)
             if k not in identity_ks and valid[:, k].any()]
    nsp = len(sp_ks)

    # tokens per (core, k): dsts sorted ascending (and srcs as well)
    toks = {}
    for c in range(NCORES):
        for ki, k in enumerate(sp_ks):
            dsts = np.nonzero(valid[c, k])[0]
            srcs = loc[c, k][dsts]
            toks[(c, ki)] = (dsts, srcs)

    # per-(block, k) run length = max count over cores
    runlen = np.zeros((NBLK, nsp), dtype=np.int64)
    for c in range(NCORES):
        for ki in range(nsp):
            d, _ = toks[(c, ki)]
            b = d // BLK
            cnt = np.bincount(b, minlength=NBLK)
            runlen[:, ki] = np.maximum(runlen[:, ki], cnt)

    # layout: blocks in order; runs inside; block group padded to 128-mult
    runs = []                     # (ki, col0, ln)  covering [0, mpad) exactly
    boff = np.zeros(NBLK + 1, dtype=np.int64)
    chunk_block = []              # dst block of each 128-token chunk
    cursor = 0
    for b in range(NBLK):
        boff[b] = cursor
        bstart = cursor
        last = None
        for ki in range(nsp):
            ln = int(runlen[b, ki])
            if ln:
                runs.append([ki, cursor, ln])
                last = len(runs) - 1
                cursor += ln
        bcnt = cursor - bstart
        bpad = -(-bcnt // P) * P
        if bpad > bcnt:
            if last is None:
                runs.append([0, cursor, bpad - bcnt])
            else:
                runs[last][2] += bpad - bcnt
            cursor = bstart + bpad
        chunk_block.extend([b] * (bpad // P))
    boff[NBLK] = cursor
    mpad = cursor
    nch = mpad // P
    runs = [tuple(r) for r in runs]

    # y-stages: consecutive blocks, sum of padded sizes <= 1024 cols
    ystages = []                  # (col0, ln)
    s0 = 0
    for b in range(NBLK + 1):
        if b == NBLK or boff[b + 1] - s0 > DSTG:
            if boff[b] > s0:
                ystages.append((int(s0), int(boff[b] - s0)))
                s0 = boff[b]
    ystages = [t for t in ystages if t[1] > 0]

    # per-core streams
    src_g = np.zeros((NCORES, mpad), dtype=np.int64)     # src row (pad: 0)
    gmask = np.zeros((NCORES, mpad), dtype=bool)
    dstpos = np.full((NCORES, mpad), -1.0, dtype=np.float32)   # dst % 512
    dstval = np.full((NCORES, mpad), -1.0, dtype=np.float32)
    srcval = np.full((NCORES, mpad), -2.0, dtype=np.float32)
    for c in range(NCORES):
        fill = {}
        for ki in range(nsp):
            d, s = toks[(c, ki)]
            b = d // BLK
            for blk in range(NBLK):
                m = b == blk
                fill.setdefault((blk, ki), (d[m], s[m]))
        for (ki, col0, ln) in runs:
            # which block is col0 in?
            blk = int(np.searchsorted(boff[1:NBLK + 1], col0, side="right"))
            d, s = fill.get((blk, ki), (np.empty(0, np.int64),) * 2)
            cnt = len(d)
            if cnt > ln:
                return None
            src_g[c, col0:col0 + cnt] = s
            gmask[c, col0:col0 + cnt] = True
            dstpos[c, col0:col0 + cnt] = (d % BLK).astype(np.float32)
            dstval[c, col0:col0 + cnt] = d.astype(np.float32)
            srcval[c, col0:col0 + cnt] = s.astype(np.float32)

    # route cells: (c1, c2) chunk pairs with any dst(t1) == src(t2) match
    cells = set()
    for c in range(NCORES):
        dv = dstval[c].reshape(nch, P)
        sv = srcval[c].reshape(nch, P)
        for c1 in range(nch):
            d1 = dv[c1][dv[c1] >= 0]
            if not len(d1):
                continue
            for c2 in range(nch):
                if np.isin(sv[c2], d1).any():
                    cells.add((c1, c2))
    cells = sorted(cells)

    # scatter one-hot [128, nch, 512] and route one-hot [128, ncell, 128]
    import ml_dtypes
    soh = np.zeros((NCORES, P, nch, BLK), dtype=ml_dtypes.float8_e4m3)
    pos_i = np.arange(BLK, dtype=np.float32)
    for c in range(NCORES):
        dp = dstpos[c].reshape(nch, P)
        for ch in range(nch):
            m = dp[ch] >= 0
            soh[c, m, ch, :] = (dp[ch][m][:, None] ==
                                pos_i[None, :]).astype(ml_dtypes.float8_e4m3)
    roh = np.zeros((NCORES, P, max(1, len(cells)), P),
                   dtype=ml_dtypes.float8_e4m3)
    for c in range(NCORES):
        dv = dstval[c].reshape(nch, P)
        sv = srcval[c].reshape(nch, P)
        for ci, (c1, c2) in enumerate(cells):
            roh[c, :, ci, :] = (dv[c1][:, None] ==
                                sv[c2][None, :]).astype(ml_dtypes.float8_e4m3)

    return dict(identity_ks=identity_ks, sp_ks=sp_ks, runs=runs,
                mpad=mpad, nch=nch, chunk_block=chunk_block,
                ystages=ystages, cells=cells,
                src_g=src_g, gmask=gmask, soh=soh, roh=roh)


# --------------------------------------------------------------------------
# device program
# --------------------------------------------------------------------------

def _build_nc(runs, mpad, nch, chunk_block, ystages, cells, nsp):
    import concourse.bacc as bacc
    import concourse.tile as tile
    from concourse import mybir

    FP16 = mybir.dt.float16
    FP32 = mybir.dt.float32
    FP8 = mybir.dt.float8e4
    Lrelu = mybir.ActivationFunctionType.Lrelu
    Copy = mybir.ActivationFunctionType.Copy
    Sqrt = mybir.ActivationFunctionType.Sqrt
    CVAR = float(PER) / float(PER - 1)
    AT = mybir.AluOpType

    ncell = max(1, len(cells))
    # chunks of each dst block
    block_chunks = [[] for _ in range(NBLK)]
    for ch, b in enumerate(chunk_block):
        block_chunks[b].append(ch)
    # route cells grouped by the y-stage containing the TARGET chunk c2
    def ystage_of_col(col):
        for si, (c0, ln) in enumerate(ystages):
            if c0 <= col < c0 + ln:
                return si
        raise AssertionError(col)

    cells_by_stage = [[] for _ in ystages]
    for ci, (c1, c2) in enumerate(cells):
        cells_by_stage[ystage_of_col(c2 * P)].append((ci, c1, c2))

    # runs grouped by y-stage
    runs_by_stage = [[] for _ in ystages]
    for (ki, c0, ln) in runs:
        runs_by_stage[ystage_of_col(c0)].append((ki, c0, ln))

    nc = bacc.Bacc(None, target_bir_lowering=False, debug=False,
                   num_swdge_queues=4)
    with tile.TileContext(nc) as tc:
        with tc.tile_pool(name="sing", bufs=1) as sing, \
             tc.tile_pool(name="big", bufs=1) as big, \
             tc.tile_pool(name="psa", bufs=3, space="PSUM") as psa, \
             tc.tile_pool(name="psb", bufs=2, space="PSUM") as psb:

            ftT = nc.dram_tensor("ftT", [P, PER], FP16, kind="ExternalInput")[:]
            g1 = nc.dram_tensor("g1", [P, mpad], FP16, kind="ExternalInput")[:]
            wsp1 = nc.dram_tensor("wsp1", [P, nsp, P], FP16,
                                  kind="ExternalInput")[:]
            wsp2 = nc.dram_tensor("wsp2", [P, nsp, P], FP16,
                                  kind="ExternalInput")[:]
            w_id1 = nc.dram_tensor("w_id1", [P, P], FP16, kind="ExternalInput")[:]
            w_id2 = nc.dram_tensor("w_id2", [P, P], FP16, kind="ExternalInput")[:]
            ident = nc.dram_tensor("ident", [P, P], FP16, kind="ExternalInput")[:]
            soh = nc.dram_tensor("soh", [P, nch, BLK], FP8,
                                 kind="ExternalInput")[:]
            roh = nc.dram_tensor("roh", [P, ncell, P], FP8,
                                 kind="ExternalInput")[:]
            gam1 = nc.dram_tensor("gam1", [P, 1], FP32, kind="ExternalInput")[:]
            bet1 = nc.dram_tensor("bet1", [P, 1], FP32, kind="ExternalInput")[:]
            gam2 = nc.dram_tensor("gam2", [P, 1], FP32, kind="ExternalInput")[:]
            bet2 = nc.dram_tensor("bet2", [P, 1], FP32, kind="ExternalInput")[:]
            out_ft = nc.dram_tensor("out_ft", [P, PER], FP16,
                                    kind="ExternalOutput")[:]

            # ---- loads (critical-path order) ----
            g1_sb = sing.tile([P, mpad], FP16, tag="g1")
            nc.sync.dma_start(g1_sb[:], g1)
            wsp_sb = [sing.tile([P, nsp, P], FP16, name=f"wsp{i}", tag=f"wsp{i}")
                      for i in range(2)]
            nc.sync.dma_start(wsp_sb[0][:], wsp1)
            w_id_sb = [sing.tile([P, P], FP16, name=f"wid{i}", tag=f"wid{i}")
                       for i in range(2)]
            nc.sync.dma_start(w_id_sb[0][:], w_id1)
            ident_sb = sing.tile([P, P], FP16, tag="ident")
            nc.sync.dma_start(ident_sb[:], ident)
            ft_sb = sing.tile([P, PER], FP16, tag="ft")
            for q in range(4):
                nc.sync.dma_start(ft_sb[:, q * 2048:(q + 1) * 2048],
                                  ftT[:, q * 2048:(q + 1) * 2048])
            soh_sb = sing.tile([P, nch, BLK], FP8, tag="soh")
            nc.sync.dma_start(soh_sb[:], soh)
            roh_sb = sing.tile([P, ncell, P], FP8, tag="roh")
            nc.sync.dma_start(roh_sb[:], roh)
            gb = []
            for i, t in enumerate([gam1, bet1, gam2, bet2]):
                s = sing.tile([P, 1], FP32, name=f"gb{i}", tag=f"gb{i}")
                nc.sync.dma_start(s[:], t)
                gb.append(s)
            eps_sb = sing.tile([P, 1], FP32, tag="eps")
            nc.vector.memset(eps_sb[:], EPS)
            nc.sync.dma_start(wsp_sb[1][:], wsp2)
            nc.sync.dma_start(w_id_sb[1][:], w_id2)

            def norm_params(mv, i):
                """inorm scale/bias from bn_aggr output mv [P,2]."""
                std = big.tile([P, 1], FP32, name=f"std{i}", tag=f"std{i}")
                nc.scalar.activation(out=std[:], in_=mv[:, 1:2], func=Sqrt,
                                     bias=eps_sb[:], scale=CVAR)
                rstd = big.tile([P, 1], FP32, name=f"rstd{i}", tag=f"rstd{i}")
                nc.vector.reciprocal(out=rstd[:], in_=std[:])
                s_ch = big.tile([P, 1], FP32, name=f"sch{i}", tag=f"sch{i}")
                nc.vector.tensor_mul(s_ch[:], gb[2 * i][:], rstd[:])
                b_ch = big.tile([P, 1], FP32, name=f"bch{i}", tag=f"bch{i}")
                nc.vector.ln_bwd_dx(b_ch[:], gb[2 * i + 1][:], mv[:, 0:1],
                                    s_ch[:], 0.0, 1.0)
                return s_ch, b_ch

            def sparse_y(i, G, gact):
                """W matmuls + transpose for conv i; returns y rows tile.

                G: feature-major token stream source tile ([P, mpad]).
                gact: None for conv1; (s,b) to apply lrelu-affine when
                      producing the matmul input from raw G (conv2).
                """
                yft = big.tile([P, mpad], FP16, name=f"yft{i}", tag=f"yft{i}")
                for si, (c0, ln) in enumerate(ystages):
                    yp = psa.tile([P, ln], FP32, tag="psa")
                    for (ki, rc0, rln) in runs_by_stage[si]:
                        # split at tile-relative 512 boundaries (PSUM bank)
                        o0 = rc0 - c0
                        while rln > 0:
                            seg = min(rln, 512 - o0 % 512)
                            nc.tensor.matmul(
                                out=yp[:, o0:o0 + seg],
                                lhsT=wsp_sb[i][:, ki, :],
                                rhs=G[:, c0 + o0:c0 + o0 + seg],
                                start=True, stop=True)
                            o0 += seg
                            rln -= seg
                    # drain Yft (Act)
                    nc.scalar.activation(yft[:, c0:c0 + ln], yp[:], Copy)
                yrows = big.tile([P, nch, P], FP16, name=f"yr{i}", tag=f"yr{i}")
                for t0 in range(0, nch, 4):
                    t1 = min(t0 + 4, nch)
                    tp = psb.tile([P, (t1 - t0) * P], FP16, tag="psb")
                    for ch in range(t0, t1):
                        nc.tensor.transpose(
                            out=tp[:, (ch - t0) * P:(ch - t0 + 1) * P],
                            in_=yft[:, ch * P:(ch + 1) * P],
                            identity=ident_sb[:])
                    nc.vector.tensor_copy(yrows[:, t0:t1, :], tp[:])
                return yrows

            def conv_dense(i, rhs_sb, yrows, cft, stats):
                """Dense + scatter per 1024-pt stage; drain + bn_stats."""
                for s in range(NDST):
                    pt = psa.tile([P, DSTG], FP32, tag="psa")
                    for h in range(2):
                        b = 2 * s + h
                        chs = block_chunks[b]
                        nc.tensor.matmul(
                            out=pt[:, h * BLK:(h + 1) * BLK],
                            lhsT=w_id_sb[i][:],
                            rhs=rhs_sb[:, b * BLK:(b + 1) * BLK],
                            start=True, stop=not chs)
                        for j, ch in enumerate(chs):
                            nc.tensor.matmul(
                                out=pt[:, h * BLK:(h + 1) * BLK],
                                lhsT=yrows[:, ch, :],
                                rhs=soh_sb[:, ch, :],
                                start=False, stop=(j == len(chs) - 1),
                                skip_group_check=True)
                    # drain: alternate scalar/vector engines
                    dst = cft[:, s * DSTG:(s + 1) * DSTG]
                    if s % 2 == 0:
                        nc.scalar.activation(dst, pt[:], Copy)
                    else:
                        nc.vector.tensor_copy(dst, pt[:])
                    for h in range(2):
                        nc.vector.bn_stats(
                            out=stats[:, 2 * s + h, :],
                            in_=cft[:, s * DSTG + h * BLK:
                                    s * DSTG + (h + 1) * BLK])

            # =========== conv1 ===========
            y1r = sparse_y(0, g1_sb, None)
            cft1 = big.tile([P, PER], FP16, tag="cft1")
            st1 = big.tile([P, NDST * 2, 6], FP32, tag="st1")
            conv_dense(0, ft_sb, y1r, cft1, st1)
            mv1 = big.tile([P, 2], FP32, tag="mv1")
            nc.vector.bn_aggr(out=mv1[:], in_=st1[:])
            s1, b1 = norm_params(mv1, 0)

            # =========== G2 = lrelu(s1 * (W_id1^T g1 + route(Y1)) + b1) ====
            g2 = big.tile([P, mpad], FP16, tag="g2")
            for si, (c0, ln) in enumerate(ystages):
                gp = psa.tile([P, ln], FP32, tag="psa")
                # route cells grouped by 512-bank half of the stage tile
                nhalf = -(-ln // 512)
                bycell = [[] for _ in range(nhalf)]
                for (ci, c1, c2) in cells_by_stage[si]:
                    bycell[(c2 * P - c0) // 512].append((ci, c1, c2))
                for h in range(nhalf):
                    h0, h1 = h * 512, min((h + 1) * 512, ln)
                    cl = bycell[h]
                    # fuse runs of cells with the same source chunk and
                    # consecutive targets (ci are consecutive too: cells are
                    # emitted sorted by (c1, c2)) into one strided matmul
                    groups = []
                    for (ci, c1, c2) in cl:
                        g = groups[-1] if groups else None
                        if (g and g[1] == c1 and c2 == g[2] + g[3]
                                and ci == g[0] + g[3]):
                            g[3] += 1
                        else:
                            groups.append([ci, c1, c2, 1])
                    nc.tensor.matmul(out=gp[:, h0:h1], lhsT=w_id_sb[0][:],
                                     rhs=g1_sb[:, c0 + h0:c0 + h1],
                                     start=True, stop=not groups)
                    for j, (ci, c1, c2, g) in enumerate(groups):
                        nc.tensor.matmul(
                            out=gp[:, c2 * P - c0:(c2 + g) * P - c0],
                            lhsT=y1r[:, c1, :],
                            rhs=roh_sb[:, ci:ci + g, :],
                            start=False, stop=(j == len(groups) - 1),
                            skip_group_check=True)
                nc.scalar.activation(out=g2[:, c0:c0 + ln], in_=gp[:],
                                     func=Lrelu, bias=b1[:], scale=s1[:],
                                     alpha=NEG_SLOPE)

            # =========== a1 = lrelu(s1*cft1 + b1) (needed for conv2 dense) ==
            a1 = big.tile([P, PER], FP16, tag="a1")
            for s in range(NDST):
                sl = slice(s * DSTG, (s + 1) * DSTG)
                nc.scalar.activation(out=a1[:, sl], in_=cft1[:, sl],
                                     func=Lrelu, bias=b1[:], scale=s1[:],
                                     alpha=NEG_SLOPE)

            # =========== conv2 ===========
            y2r = sparse_y(1, g2, None)
            cft2 = big.tile([P, PER], FP16, tag="cft2")
            st2 = big.tile([P, NDST * 2, 6], FP32, tag="st2")
            conv_dense(1, a1, y2r, cft2, st2)
            mv2 = big.tile([P, 2], FP32, tag="mv2")
            nc.vector.bn_aggr(out=mv2[:], in_=st2[:])
            s2, b2 = norm_params(mv2, 1)

            # =========== tail: out = lrelu(s2*cft2 + ft + b2) ===========
            osb = big.tile([P, PER], FP16, tag="osb")
            for s in range(NDST):
                sl = slice(s * DSTG, (s + 1) * DSTG)
                z = big.tile([P, DSTG], FP16, name=f"z{s}", tag=f"z{s}")
                nc.vector.scalar_tensor_tensor(
                    out=z[:], in0=cft2[:, sl], scalar=s2[:],
                    in1=ft_sb[:, sl], op0=AT.mult, op1=AT.add)
                nc.scalar.activation(out=osb[:, sl], in_=z[:],
                                     func=Lrelu, bias=b2[:], scale=1.0,
                                     alpha=NEG_SLOPE)
                nc.sync.dma_start(out_ft[:, sl], osb[:, sl])

    nc.compile()
    return nc


# --------------------------------------------------------------------------
# numpy fallback (only used if sharding assumptions fail)
# --------------------------------------------------------------------------

def _numpy_ref(feats, batch_ids, neighbor_idx, w1, gamma1, beta1,
               w2, gamma2, beta2):
    f = feats.astype(np.float64)

    def conv(x, w):
        out = np.zeros((x.shape[0], w.shape[-1]), dtype=np.float64)
        for k in range(KVOL):
            idx = neighbor_idx[k]
            g = np.where((idx >= 0)[:, None], x[np.maximum(idx, 0)], 0.0)
            out += g @ w[k]
        return out

    def inorm(x, gamma, beta):
        out = np.empty_like(x)
        for b in range(B):
            m = batch_ids == b
            xb = x[m]
            cnt = xb.shape[0]
            mean = xb.mean(axis=0)
            var = ((xb * xb).sum(0) - cnt * mean * mean) / (cnt - 1.0) + EPS
            out[m] = (xb - mean) / np.sqrt(var)
        return out * gamma + beta

    def leaky(x):
        return np.where(x >= 0, x, NEG_SLOPE * x)

    out = leaky(inorm(conv(f, w1.astype(np.float64)), gamma1, beta1))
    out = inorm(conv(out, w2.astype(np.float64)), gamma2, beta2)
    out = leaky(out + f)
    return out.astype(np.float32)


# --------------------------------------------------------------------------
# entry point
# --------------------------------------------------------------------------

def kernel(feats, batch_ids, neighbor_idx, w1, gamma1, beta1,
           w2, gamma2, beta2):
    feats = np.asarray(feats, dtype=np.float32)
    batch_ids = np.asarray(batch_ids)
    neighbor_idx = np.asarray(neighbor_idx)
    w1 = np.asarray(w1, dtype=np.float32)
    w2 = np.asarray(w2, dtype=np.float32)
    gamma1 = np.asarray(gamma1, dtype=np.float32).reshape(-1)
    beta1 = np.asarray(beta1, dtype=np.float32).reshape(-1)
    gamma2 = np.asarray(gamma2, dtype=np.float32).reshape(-1)
    beta2 = np.asarray(beta2, dtype=np.float32).reshape(-1)

    ok = (feats.shape == (N, C) and neighbor_idx.shape == (KVOL, N)
          and np.array_equal(batch_ids,
                             np.repeat(np.arange(B, dtype=batch_ids.dtype),
                                       PER)))
    plan = None
    if ok:
        key = hash(neighbor_idx.tobytes())
        if key not in _plan_cache:
            _plan_cache[key] = _build_plan(neighbor_idx)
        plan = _plan_cache[key]
    if plan is None or not plan["sp_ks"]:
        return _numpy_ref(feats, batch_ids, neighbor_idx, w1, gamma1, beta1,
                          w2, gamma2, beta2)

    runs = plan["runs"]
    mpad = plan["mpad"]
    nch = plan["nch"]
    nsp = len(plan["sp_ks"])

    pkey = (tuple(runs), mpad, tuple(plan["chunk_block"]),
            tuple(plan["ystages"]), tuple(plan["cells"]), nsp)
    if pkey not in _prog_cache:
        _prog_cache[pkey] = _build_nc(runs, mpad, nch, plan["chunk_block"],
                                      plan["ystages"], plan["cells"], nsp)
    nc = _prog_cache[pkey]

    w_id1 = np.zeros((C, C), dtype=np.float32)
    w_id2 = np.zeros((C, C), dtype=np.float32)
    for k in plan["identity_ks"]:
        w_id1 += w1[k]
        w_id2 += w2[k]
    wsp1 = np.ascontiguousarray(
        w1[plan["sp_ks"]].transpose(1, 0, 2)).astype(np.float16)
    wsp2 = np.ascontiguousarray(
        w2[plan["sp_ks"]].transpose(1, 0, 2)).astype(np.float16)

    f16 = feats.astype(np.float16)
    in_maps = []
    for c in range(NCORES):
        fc = f16[c * PER:(c + 1) * PER]          # [PER, C]
        g1v = fc[plan["src_g"][c]].T.copy()      # [C, mpad]
        g1v[:, ~plan["gmask"][c]] = 0
        m = dict(
            ftT=np.ascontiguousarray(fc.T),
            g1=g1v,
            wsp1=wsp1,
            wsp2=wsp2,
            w_id1=w_id1.astype(np.float16),
            w_id2=w_id2.astype(np.float16),
            ident=np.eye(C, dtype=np.float16),
            soh=plan["soh"][c],
            roh=plan["roh"][c],
            gam1=gamma1.reshape(C, 1),
            bet1=beta1.reshape(C, 1),
            gam2=gamma2.reshape(C, 1),
            bet2=beta2.reshape(C, 1),
        )
        in_maps.append(m)

    from concourse.bass_utils import run_bass_kernel_spmd
    res = run_bass_kernel_spmd(nc, in_maps, core_ids=list(range(NCORES)))
    global _last_results
    _last_results = res

    out = np.empty((N, C), dtype=np.float32)
    for c in range(NCORES):
        out[c * PER:(c + 1) * PER] = \
            res.results[c]["out_ft"].astype(np.float32).T
    return out
